# revision 1
# baseline (speedup 1.0000x reference)
"""Causal self-attention (RoPE, B=4 S=2048 D=2048 H=16) on 8 Trainium2 cores.

Sharding: core c = 2*b + hh  ->  batch b = c // 2, head-half hh = c % 2
(8 heads per core).  Each core computes qkv projection for its heads,
attention, and a partial output projection over its 1024 y-features;
the host sums the two partials of each batch.

All matmuls run as float32r (TF32-like, full PE rate).  Causality is
implemented structurally (key-block skipping + 4 diagonal masks); the
softmax skips max-subtraction (scores are O(1) here, exp is safe).
"""
import sys

try:
    import concourse.bass as _chk  # noqa: F401
except ImportError:
    for p in ("/opt/trn_rl_repo", "/root/.axon_site/_ro/trn_rl_repo"):
        if p not in sys.path:
            sys.path.insert(0, p)

import math
import numpy as np

import concourse.bass as bass
import concourse.tile as tile
from concourse import mybir
from concourse.bass_utils import run_bass_kernel_spmd

N_CORES = 8
B = 4
D = 2048
H = 16
HD = 128
HEADS_PER_CORE = 8
ROPE_BASE = 10000.0
F32 = mybir.dt.float32
F32R = mybir.dt.float32r
EXP = mybir.ActivationFunctionType.Exp
SCALE = 1.0 / math.sqrt(HD)


def split_ctrl_waits(nc, maxw=1):
    """Walrus in this env can't encode >1 sem-wait on many instruction
    formats; move extras onto preceding same-engine NoOps."""
    nid = [0]
    for f in nc.m.functions:
        for b in f.blocks:
            new_insts = []
            for inst in b.instructions:
                si = inst.sync_info
                if si is not None and si.on_wait is not None and len(si.on_wait) > maxw:
                    waits = list(si.on_wait)
                    while len(waits) > maxw:
                        chunk, waits = waits[:maxw], waits[maxw:]
                        nid[0] += 1
                        nop = mybir.InstNoOp(
                            name=f"I-waitsplit-{nid[0]}",
                            ins=[], outs=[],
                            sync_info=mybir.SyncInfo(on_wait=chunk, on_update=[]),
                        )
                        nop.engine = inst.engine
                        new_insts.append(nop)
                    si.on_wait = waits
                new_insts.append(inst)
            b.instructions[:] = new_insts


def build_nc(S=2048, repeat=1, stages="ABC"):
    """One SPMD program; all 8 cores run it on different data."""
    KT = D // 128            # 16 contraction tiles for projections
    NQ = S // 512            # query groups
    NK = S // 128            # key tiles / seq tiles
    QF = HEADS_PER_CORE * HD  # 1024 q (=k,=v) features per core

    nc = bass.Bass("TRN2", debug=False, num_devices=N_CORES)

    xT = nc.dram_tensor("xT", [D, S], F32, kind="ExternalInput")
    wq = nc.dram_tensor("wq", [KT, HEADS_PER_CORE, 128, 128], F32, kind="ExternalInput")
    wk = nc.dram_tensor("wk", [KT, HEADS_PER_CORE, 128, 128], F32, kind="ExternalInput")
    wv = nc.dram_tensor("wv", [KT, QF // 512, 128, 512], F32, kind="ExternalInput")
    wo = nc.dram_tensor("wo", [HEADS_PER_CORE, D // 512, 128, 512], F32, kind="ExternalInput")
    cosf = nc.dram_tensor("cosf", [128, S], F32, kind="ExternalInput")
    sinf = nc.dram_tensor("sinf", [128, S], F32, kind="ExternalInput")  # sign-folded
    dmasks = nc.dram_tensor("dmasks", [128, 4 * 512], F32, kind="ExternalInput")
    onesc = nc.dram_tensor("onesc", [128, 128], F32, kind="ExternalInput")
    out = nc.dram_tensor("out", [S, D], F32, kind="ExternalOutput")

    with tile.TileContext(nc) as tc:
        with tc.tile_pool(name="const", bufs=1) as constp, \
             tc.tile_pool(name="dram", bufs=1, space="DRAM") as dramp:
            cos_t = constp.tile([128, S], F32, name="cos_t")
            sin_t = constp.tile([128, S], F32, name="sin_t")
            mask_t = constp.tile([128, 4 * 512], F32, name="mask_t")
            ones_t = constp.tile([128, 128], F32R, name="ones_t")
            onesf = constp.tile([128, 128], F32, name="onesf")
            nc.sync.dma_start(cos_t[:], cosf[:])
            nc.sync.dma_start(sin_t[:], sinf[:])
            nc.sync.dma_start(mask_t[:], dmasks[:])
            nc.sync.dma_start(onesf[:], onesc[:])
            nc.vector.tensor_copy(ones_t[:], onesf[:])

            q_sp = dramp.tile([HEADS_PER_CORE, 128, S], F32, name="q_sp")
            k_sp = dramp.tile([HEADS_PER_CORE, 128, S], F32, name="k_sp")
            v_sp = dramp.tile([S, QF], F32, name="v_sp")

            for _rep in range(repeat):
                _body(nc, tc, S, KT, NQ, NK, QF,
                      xT, wq, wk, wv, wo, out,
                      cos_t, sin_t, mask_t, ones_t,
                      q_sp, k_sp, v_sp, stages)

    split_ctrl_waits(nc)
    return nc


def _body(nc, tc, S, KT, NQ, NK, QF,
          xT, wq, wk, wv, wo, out,
          cos_t, sin_t, mask_t, ones_t,
          q_sp, k_sp, v_sp, stages="ABC"):
    NC4 = S // 512  # moving chunks over seq

    # ---------------- Stage A: qkv projection ----------------
    with tc.tile_pool(name="xtp", bufs=1) as xtp, \
         tc.tile_pool(name="ldp", bufs=2) as ldp, \
         tc.tile_pool(name="wtp", bufs=4) as wtp, \
         tc.tile_pool(name="aop", bufs=4) as aop, \
         tc.tile_pool(name="aps", bufs=2, space="PSUM") as aps:

        # x^T resident as f32r
        xt = xtp.tile([128, KT * S], F32R, name="xt")
        for k in range(KT):
            xf = ldp.tile([128, S], F32, name=f"xf{k}", tag="xf")
            nc.sync.dma_start(xf[:], xT[k * 128:(k + 1) * 128, :])
            nc.scalar.copy(xt[:, k * S:(k + 1) * S], xf[:])

        # v natural: out[seq 128, vfeat 512] = xT-block.T @ Wv
        for st in range(NK):
            pos = [aps.tile([128, 512], F32, name=f"vpo{st}_{i}", tag=f"apo{i}")
                   for i in range(QF // 512)]
            for k in range(KT):
                for ncx in range(QF // 512):
                    wf = wtp.tile([128, 512], F32, name=f"vwf{st}_{k}_{ncx}", tag="vwf")
                    nc.sync.dma_start(wf[:], wv[k, ncx])
                    wr = wtp.tile([128, 512], F32R, name=f"vwr{st}_{k}_{ncx}", tag="vwr")
                    nc.vector.tensor_copy(wr[:], wf[:])
                    nc.tensor.matmul(
                        pos[ncx][:], xt[:, k * S + st * 128: k * S + st * 128 + 128], wr[:],
                        start=(k == 0), stop=(k == KT - 1))
            for ncx in range(QF // 512):
                ot = aop.tile([128, 512], F32R, name=f"vot{st}_{ncx}", tag="aot")
                nc.scalar.copy(ot[:], pos[ncx][:])
                nc.sync.dma_start(v_sp[st * 128:(st + 1) * 128, ncx * 512:(ncx + 1) * 512], ot[:].bitcast(F32))

        # q^T, k^T: out[feat 128, seq 512] = W[k,m].T-block @ xT
        for w_dram, spill in ((wq, q_sp), (wk, k_sp)):
            for m in range(HEADS_PER_CORE):
                pos = [aps.tile([128, 512], F32, name=f"apo{m}_{i}", tag=f"apo{i}")
                       for i in range(NC4)]
                for k in range(KT):
                    wf = wtp.tile([128, 128], F32, name=f"wf{m}_{k}", tag="wf")
                    nc.sync.dma_start(wf[:], w_dram[k, m])
                    wr = wtp.tile([128, 128], F32R, name=f"wr{m}_{k}", tag="wr")
                    nc.vector.tensor_copy(wr[:], wf[:])
                    for ncx in range(NC4):
                        nc.tensor.matmul(
                            pos[ncx][:], wr[:], xt[:, k * S + ncx * 512: k * S + ncx * 512 + 512],
                            start=(k == 0), stop=(k == KT - 1))
                for ncx in range(NC4):
                    ot = aop.tile([128, 512], F32R, name=f"aot{m}_{ncx}", tag="aot")
                    nc.scalar.copy(ot[:], pos[ncx][:])
                    nc.sync.dma_start(spill[m, :, ncx * 512:(ncx + 1) * 512], ot[:].bitcast(F32))

    if stages == "A":
        # debug: dump spills into out
        with tc.tile_pool(name="dbg", bufs=2) as dbg:
            for (src, r0) in ((q_sp[0], 0), (k_sp[0], 1)):
                t = dbg.tile([128, S], F32, name=f"dbgq{r0}", tag="dbg")
                nc.sync.dma_start(t[:], src)
                nc.sync.dma_start(out[r0 * 128:(r0 + 1) * 128, 0:S], t[:])
            t = dbg.tile([128, QF], F32, name="dbgv", tag="dbgv")
            nc.sync.dma_start(t[:], v_sp[0:128, :])
            nc.sync.dma_start(out[2 * 128:3 * 128, 0:QF], t[:])
        return

    # ---------------- Stage B: attention ----------------
    with tc.tile_pool(name="ybuf", bufs=1) as yp:
      yhat = yp.tile([128, HEADS_PER_CORE * S], F32R, name="yhat")
      with tc.tile_pool(name="bh1", bufs=1) as bh1, \
         tc.tile_pool(name="bh2", bufs=2) as bh2, \
         tc.tile_pool(name="ptp", bufs=3) as ptp, \
         tc.tile_pool(name="bps", bufs=2, space="PSUM") as bps:

        for h in range(HEADS_PER_CORE):
            qf = bh2.tile([128, S], F32, name=f"qf{h}", tag="qf")
            nc.sync.dma_start(qf[:], q_sp[h])
            kf = bh2.tile([128, S], F32, name=f"kf{h}", tag="kf")
            nc.sync.dma_start(kf[:], k_sp[h])
            vsb = bh2.tile([128, S], F32R, name=f"vsb{h}", tag="vsb")
            for kt in range(NK):
                vf = bh2.tile([128, 128], F32, name=f"vf{h}_{kt}", tag="vf")
                nc.sync.dma_start(vf[:], v_sp[kt * 128:(kt + 1) * 128, h * 128:(h + 1) * 128])
                nc.vector.tensor_copy(vsb[:, kt * 128:(kt + 1) * 128], vf[:])

            def rope(dst, src, tagp):
                sw = bh1.tile([128, S], F32, name=f"sw_{tagp}{h}", tag="sw")
                nc.vector.tensor_copy(sw[0:64, :], src[64:128, :])
                nc.vector.tensor_copy(sw[64:128, :], src[0:64, :])
                nc.vector.tensor_mul(src[:], src[:], cos_t[:])
                nc.vector.tensor_mul(sw[:], sw[:], sin_t[:])
                nc.vector.tensor_add(dst[:], src[:], sw[:])

            qr = bh1.tile([128, S], F32R, name=f"qr{h}", tag="qr")
            rope(qr, qf, "q")
            kr = bh1.tile([128, S], F32R, name=f"kr{h}", tag="kr")
            rope(kr, kf, "k")

            for qg in range(NQ):
                yps = bps.tile([128, 512], F32, name=f"yps{h}_{qg}", tag="yps")
                dps = bps.tile([128, 512], F32, name=f"dps{h}_{qg}", tag="dps")
                nkt = 4 * qg + 4
                for kt in range(nkt):
                    sps = bps.tile([128, 512], F32, name=f"sps{h}_{qg}_{kt}", tag="sps")
                    nc.tensor.matmul(sps[:], kr[:, kt * 128:(kt + 1) * 128],
                                     qr[:, qg * 512:(qg + 1) * 512],
                                     start=True, stop=True)
                    pt = ptp.tile([128, 512], F32R, name=f"pt{h}_{qg}_{kt}", tag="pt")
                    nc.scalar.activation(pt[:], sps[:], EXP, scale=SCALE)
                    j = kt - 4 * qg
                    if j >= 0:  # diagonal block: apply causal mask
                        ptm = ptp.tile([128, 512], F32R, name=f"ptm{h}_{qg}_{kt}", tag="ptm")
                        nc.vector.tensor_mul(ptm[:], pt[:], mask_t[:, j * 512:(j + 1) * 512])
                        pt = ptm
                    nc.tensor.matmul(yps[:], vsb[:, kt * 128:(kt + 1) * 128], pt[:],
                                     start=(kt == 0), stop=(kt == nkt - 1))
                    nc.tensor.matmul(dps[:], ones_t[:], pt[:],
                                     start=(kt == 0), stop=(kt == nkt - 1))
                rec = bh2.tile([128, 512], F32, name=f"rec{h}_{qg}", tag="rec")
                nc.vector.reciprocal(rec[:], dps[:])
                nc.vector.tensor_mul(
                    yhat[:, h * S + qg * 512: h * S + qg * 512 + 512], yps[:], rec[:])

      if stages == "AB":
          with tc.tile_pool(name="dbg2", bufs=2) as dbg:
              for r0 in range(2):
                  t = dbg.tile([128, S], F32, name=f"dbgy{r0}", tag="dbg")
                  nc.vector.tensor_copy(t[:], yhat[:, r0 * S:(r0 + 1) * S])
                  nc.sync.dma_start(out[r0 * 128:(r0 + 1) * 128, 0:S], t[:])
          return

      # ---------------- Stage C: output projection (partial) ----------------
      with tc.tile_pool(name="wop", bufs=1) as wop, \
             tc.tile_pool(name="wol", bufs=2) as wol, \
             tc.tile_pool(name="cop", bufs=4) as cop, \
             tc.tile_pool(name="cps", bufs=2, space="PSUM") as cps:

        wo_sb = wop.tile([128, HEADS_PER_CORE * 4 * 512], F32R, name="wo_sb")
        for h in range(HEADS_PER_CORE):
            for oc in range(4):
                wf = wol.tile([128, 512], F32, name=f"wof{h}_{oc}", tag="wof")
                nc.sync.dma_start(wf[:], wo[h, oc])
                nc.vector.tensor_copy(
                    wo_sb[:, (h * 4 + oc) * 512:(h * 4 + oc + 1) * 512], wf[:])

        for st in range(NK):
            pos = [cps.tile([128, 512], F32, name=f"cpo{st}_{i}", tag=f"cpo{i}")
                   for i in range(4)]
            for h in range(HEADS_PER_CORE):
                for oc in range(4):
                    nc.tensor.matmul(
                        pos[oc][:],
                        yhat[:, h * S + st * 128: h * S + st * 128 + 128],
                        wo_sb[:, (h * 4 + oc) * 512:(h * 4 + oc + 1) * 512],
                        start=(h == 0), stop=(h == HEADS_PER_CORE - 1))
            for oc in range(4):
                ot = cop.tile([128, 512], F32, name=f"cot{st}_{oc}", tag="cot")
                nc.scalar.copy(ot[:], pos[oc][:])
                nc.sync.dma_start(out[st * 128:(st + 1) * 128, oc * 512:(oc + 1) * 512], ot[:])


def prep_in_maps(x, positions, Wqkv, Wout, S=2048):
    """Host-side shard/format. Returns per-core input dicts."""
    KT = D // 128
    QF = HEADS_PER_CORE * HD

    # RoPE tables from positions (deinterleaved pair layout, sign-folded sin)
    inv_freq = 1.0 / (ROPE_BASE ** (np.arange(0, HD, 2, dtype=np.float64) / HD))  # [64]
    pos = np.asarray(positions).astype(np.float64)[:S]
    freq = pos[None, :] * inv_freq[:, None]          # [64, S]
    c = np.cos(freq).astype(np.float32)
    s = np.sin(freq).astype(np.float32)
    cosf = np.vstack([c, c])                          # [128, S]
    sinf = np.vstack([-s, s])                         # [128, S]

    # diagonal causal masks M_j [128, 4*512]: key r (partition), query col c;
    # block j: cols [0,128j) dead, [128j,128j+128) triu (r<=c-128j), rest live
    dm = np.zeros((128, 4, 512), np.float32)
    for j in range(4):
        dm[:, j, 128 * j:128 * (j + 1)] = np.triu(np.ones((128, 128), np.float32))
        dm[:, j, 128 * (j + 1):] = 1.0
    dmasks = dm.reshape(128, 4 * 512)

    onesc = np.ones((128, 128), np.float32)

    # per-head even/odd column permutation for q,k weights
    perm = np.concatenate([np.arange(0, HD, 2), np.arange(1, HD, 2)])

    in_maps = []
    for c_id in range(N_CORES):
        b, hh = c_id // 2, c_id % 2
        xTb = np.ascontiguousarray(x[b, :S, :].T)     # [D, S]
        f0 = hh * QF
        Wq = Wqkv[:, f0:f0 + QF]
        Wk = Wqkv[:, D + f0:D + f0 + QF]
        Wv = Wqkv[:, 2 * D + f0:2 * D + f0 + QF]
        # permute within each head for q, k
        Wqp = Wq.reshape(D, HEADS_PER_CORE, HD)[:, :, perm].reshape(D, QF)
        Wkp = Wk.reshape(D, HEADS_PER_CORE, HD)[:, :, perm].reshape(D, QF)
        # pack [KT, m, 128, 128]
        wq_p = np.ascontiguousarray(
            Wqp.reshape(KT, 128, HEADS_PER_CORE, 128).transpose(0, 2, 1, 3))
        wk_p = np.ascontiguousarray(
            Wkp.reshape(KT, 128, HEADS_PER_CORE, 128).transpose(0, 2, 1, 3))
        wv_p = np.ascontiguousarray(
            Wv.reshape(KT, 128, QF // 512, 512).transpose(0, 2, 1, 3))
        Woh = Wout[f0:f0 + QF, :]                     # [1024, 2048]
        wo_p = np.ascontiguousarray(
            Woh.reshape(HEADS_PER_CORE, 128, 4, 512).transpose(0, 2, 1, 3))
        in_maps.append({
            "xT": xTb, "wq": wq_p, "wk": wk_p, "wv": wv_p, "wo": wo_p,
            "cosf": cosf, "sinf": sinf, "dmasks": dmasks, "onesc": onesc,
        })
    return in_maps


def kernel(x, positions, mask, Wqkv, Wout):
    x = np.asarray(x, dtype=np.float32)
    Wqkv = np.asarray(Wqkv, dtype=np.float32)
    Wout = np.asarray(Wout, dtype=np.float32)
    S = x.shape[1]
    nc = build_nc(S=S)
    in_maps = prep_in_maps(x, positions, Wqkv, Wout, S=S)
    res = run_bass_kernel_spmd(nc, in_maps, core_ids=list(range(N_CORES)))
    outs = [res.results[c]["out"] for c in range(N_CORES)]
    full = np.stack([outs[2 * b] + outs[2 * b + 1] for b in range(B)], axis=0)
    return full.astype(np.float32)



# revision 8
# speedup vs baseline: 1.1184x; 1.1184x over previous
"""Causal self-attention (RoPE, B=4 S=2048 D=2048 H=16) on 8 Trainium2 cores.

Sharding: core c = 2*b + hh  ->  batch b = c // 2, head-half hh = c % 2
(8 heads per core).  Each core computes qkv projection for its heads,
attention, and a partial output projection over its 1024 y-features;
the host sums the two partials of each batch.

v2: fully SBUF-resident (no DRAM spills), fp16 inputs/compute with f32
PSUM accumulation, RoPE fused into PSUM eviction, causal diagonal-block
column trimming, software-pipelined attention inner loop.
"""
import sys

try:
    import concourse.bass as _chk  # noqa: F401
except ImportError:
    for p in ("/opt/trn_rl_repo", "/root/.axon_site/_ro/trn_rl_repo"):
        if p not in sys.path:
            sys.path.insert(0, p)

import math
import numpy as np
import ml_dtypes

import concourse.bass as bass
import concourse.tile as tile
from concourse import mybir
from concourse.bass_utils import run_bass_kernel_spmd

N_CORES = 8
B = 4
D = 2048
H = 16
HD = 128
M = 8                     # heads per core
QF = M * HD               # 1024 q/k/v features per core
KT = D // 128             # 16 contraction tiles
SH = 1024                 # phase-1 seq chunk
ROPE_BASE = 10000.0
F32 = mybir.dt.float32
F16 = mybir.dt.float16
NPF16 = np.float16
EXP = mybir.ActivationFunctionType.Exp
SCALE = 1.0 / math.sqrt(HD)


def split_ctrl_waits(nc, maxw=1):
    """Walrus in this env can't encode >1 sem-wait on many instruction
    formats; move extras onto preceding same-engine NoOps."""
    nid = [0]
    for f in nc.m.functions:
        for b in f.blocks:
            new_insts = []
            for inst in b.instructions:
                si = inst.sync_info
                if si is not None and si.on_wait is not None and len(si.on_wait) > maxw:
                    waits = list(si.on_wait)
                    while len(waits) > maxw:
                        chunk, waits = waits[:maxw], waits[maxw:]
                        nid[0] += 1
                        nop = mybir.InstNoOp(
                            name=f"I-waitsplit-{nid[0]}",
                            ins=[], outs=[],
                            sync_info=mybir.SyncInfo(on_wait=chunk, on_update=[]),
                        )
                        nop.engine = inst.engine
                        new_insts.append(nop)
                    si.on_wait = waits
                new_insts.append(inst)
            b.instructions[:] = new_insts


def build_nc(S=2048, repeat=1, stages="ABC", split_waits=True):
    """One SPMD program; all 8 cores run it on different data."""
    nc = bass.Bass("TRN2", debug=False, num_devices=N_CORES)

    xT = nc.dram_tensor("xT", [D, S], F16, kind="ExternalInput")
    wq = nc.dram_tensor("wq", [M, 128, KT * 128], F16, kind="ExternalInput")
    wk = nc.dram_tensor("wk", [M, 128, KT * 128], F16, kind="ExternalInput")
    wv = nc.dram_tensor("wv", [2, 128, KT * 512], F16, kind="ExternalInput")
    wo = nc.dram_tensor("wo", [128, M * 4 * 512], F16, kind="ExternalInput")
    cosf = nc.dram_tensor("cosf", [128, S], F16, kind="ExternalInput")
    sinf = nc.dram_tensor("sinf", [128, S], F16, kind="ExternalInput")  # pre-swapped+sign-folded
    dmasks = nc.dram_tensor("dmasks", [128, 4 * 512], F16, kind="ExternalInput")
    out = nc.dram_tensor("out", [S, D], F32, kind="ExternalOutput")

    with tile.TileContext(nc) as tc:
        with tc.tile_pool(name="const", bufs=1) as constp:
            cos_t = constp.tile([128, S], F16, name="cos_t")
            sin_t = constp.tile([128, S], F16, name="sin_t")
            mask_t = constp.tile([128, 4 * 512], F16, name="mask_t")
            ones_t = constp.tile([128, 128], F16, name="ones_t")
            nc.sync.dma_start(cos_t[:], cosf[:])
            nc.sync.dma_start(sin_t[:], sinf[:])
            nc.sync.dma_start(mask_t[:], dmasks[:])
            nc.vector.memset(ones_t[:], 1.0)

            qr = constp.tile([128, M * S], F16, name="qr")
            kr = constp.tile([128, M * S], F16, name="kr")
            vsb = constp.tile([128, (S // 128) * QF], F16, name="vsb")
            # vsb col = st*1024 + f   (f = m*128 + hd)

            for _rep in range(repeat):
                _body(nc, tc, S, xT, wq, wk, wv, wo, out,
                      cos_t, sin_t, mask_t, ones_t, qr, kr, vsb, stages)

    if split_waits:
        split_ctrl_waits(nc)
    return nc


def _phase1(nc, tc, S, xT, wq, wk, wv, cos_t, sin_t, qr, kr, vsb):
    """QKV projection + fused RoPE, fp16 in, f32 psum, fp16 out."""
    NH = S // SH              # seq halves
    NCX = SH // 512           # 512-chunks per half

    with tc.tile_pool(name="p1", bufs=2) as xp, \
         tc.tile_pool(name="p1wv", bufs=1) as wvp, \
         tc.tile_pool(name="p1w", bufs=2) as wp, \
         tc.tile_pool(name="p1u", bufs=3) as up, \
         tc.tile_pool(name="p1ps", bufs=2, space="PSUM") as psp:

        for h in range(NH):
            xt = xp.tile([128, KT * SH], F16, name=f"xt{h}", tag="xt")
            for k in range(KT):
                nc.sync.dma_start(
                    xt[:, k * SH:(k + 1) * SH],
                    xT[k * 128:(k + 1) * 128, h * SH:(h + 1) * SH])

            # ---- q^T, k^T with fused RoPE (first: small weight loads) ----
            for w_dram, dst, tg in ((wq, qr, "q"), (wk, kr, "k")):
                for m in range(M):
                    wt = wp.tile([128, KT * 128], F16, name=f"w{tg}{h}_{m}", tag="wqk")
                    nc.sync.dma_start(wt[:], w_dram[m])
                    pos = [psp.tile([128, 512], F32, name=f"qp{tg}{h}_{m}_{c}", tag=f"qps{c}")
                           for c in range(NCX)]
                    for k in range(KT):
                        for c in range(NCX):
                            nc.tensor.matmul(
                                pos[c][:],
                                wt[:, k * 128:(k + 1) * 128],
                                xt[:, k * SH + c * 512: k * SH + c * 512 + 512],
                                start=(k == 0), stop=(k == KT - 1))
                    for c in range(NCX):
                        s0 = h * SH + c * 512     # seq offset
                        dslot = dst[:, m * S + s0: m * S + s0 + 512]
                        u = up.tile([128, 512], F16, name=f"u{tg}{h}_{m}_{c}", tag="u")
                        us = up.tile([128, 512], F16, name=f"us{tg}{h}_{m}_{c}", tag="us")
                        nc.vector.tensor_mul(u[:], pos[c][:], sin_t[:, s0:s0 + 512])
                        nc.vector.tensor_mul(dslot, pos[c][:], cos_t[:, s0:s0 + 512])
                        nc.vector.tensor_copy(us[0:64, :], u[64:128, :])
                        nc.vector.tensor_copy(us[64:128, :], u[0:64, :])
                        nc.vector.tensor_add(dslot, dslot, us[:])

            # ---- v: natural [seq, feat] ----
            for ncx in range(2):
                wvt = wvp.tile([128, KT * 512], F16, name=f"wv{h}_{ncx}", tag="wv")
                nc.sync.dma_start(wvt[:], wv[ncx])
                for st in range(SH // 128):
                    ps = psp.tile([128, 512], F32, name=f"vp{h}_{ncx}_{st}", tag="vps")
                    for k in range(KT):
                        nc.tensor.matmul(
                            ps[:],
                            xt[:, k * SH + st * 128: k * SH + st * 128 + 128],
                            wvt[:, k * 512:(k + 1) * 512],
                            start=(k == 0), stop=(k == KT - 1))
                    stg = h * (SH // 128) + st
                    nc.scalar.copy(vsb[:, stg * QF + ncx * 512: stg * QF + ncx * 512 + 512], ps[:])


def _phase23(nc, tc, S, wo, out, mask_t, ones_t, qr, kr, vsb, stages):
    NQ = S // 512
    with tc.tile_pool(name="p2", bufs=1) as cp:
        yhat = cp.tile([128, M * S], F16, name="yhat")
        wo_sb = cp.tile([128, M * 4 * 512], F16, name="wo_sb")
        nc.sync.dma_start(wo_sb[:], wo[:])

        _phase2(nc, tc, S, mask_t, ones_t, qr, kr, vsb, yhat)

        if stages == "AB":
            with tc.tile_pool(name="dbg2", bufs=2) as dbg:
                for r0 in range(2):
                    t = dbg.tile([128, S], F32, name=f"dbgy{r0}", tag="dbg")
                    nc.vector.tensor_copy(t[:], yhat[:, r0 * S:(r0 + 1) * S])
                    nc.sync.dma_start(out[r0 * 128:(r0 + 1) * 128, 0:S], t[:])
            return

        # ---------------- phase 3: output projection (partial) ----------------
        with tc.tile_pool(name="p3o", bufs=4) as cop, \
             tc.tile_pool(name="p3ps", bufs=2, space="PSUM") as cps:
            for st in range(S // 128):
                pos = [cps.tile([128, 512], F32, name=f"cpo{st}_{i}", tag=f"cpo{i}")
                       for i in range(4)]
                for m in range(M):
                    for oc in range(4):
                        nc.tensor.matmul(
                            pos[oc][:],
                            yhat[:, m * S + st * 128: m * S + st * 128 + 128],
                            wo_sb[:, (m * 4 + oc) * 512:(m * 4 + oc + 1) * 512],
                            start=(m == 0), stop=(m == M - 1))
                for oc in range(4):
                    ot = cop.tile([128, 512], F32, name=f"cot{st}_{oc}", tag="cot")
                    nc.scalar.copy(ot[:], pos[oc][:])
                    nc.sync.dma_start(
                        out[st * 128:(st + 1) * 128, oc * 512:(oc + 1) * 512], ot[:])


def _phase2(nc, tc, S, mask_t, ones_t, qr, kr, vsb, yhat):
    NQ = S // 512
    with tc.tile_pool(name="p2pt", bufs=4) as ptp, \
         tc.tile_pool(name="p2n", bufs=2) as np_, \
         tc.tile_pool(name="p2ps", bufs=3, space="PSUM") as sps_p, \
         tc.tile_pool(name="p2ac", bufs=2, space="PSUM") as acc_p:

        for m in range(M):
            qh0 = m * S
            for qg in range(NQ):
                nkt = 4 * qg + 4
                yps = acc_p.tile([128, 512], F32, name=f"yps{m}_{qg}", tag="yps")
                dps = acc_p.tile([128, 512], F32, name=f"dps{m}_{qg}", tag="dps")

                # software-pipelined: scores/exp run LOOKAHEAD blocks ahead
                # of the yps/dps accumulation matmuls.
                LOOKAHEAD = 2
                pts = {}

                def emit_front(kt, m=m, qg=qg, nkt=nkt, yps=yps, dps=dps):
                    j = kt - 4 * qg
                    c0 = 128 * j if j >= 0 else 0
                    w = 512 - c0
                    sp = sps_p.tile([128, 512], F32, name=f"sp{m}_{qg}_{kt}", tag="sps")
                    nc.tensor.matmul(
                        sp[:, c0:512],
                        kr[:, qh0 + kt * 128: qh0 + kt * 128 + 128],
                        qr[:, qh0 + qg * 512 + c0: qh0 + qg * 512 + 512],
                        start=True, stop=True)
                    pt = ptp.tile([128, 512], F16, name=f"pt{m}_{qg}_{kt}", tag="pt")
                    nc.scalar.activation(pt[:, c0:512], sp[:, c0:512], EXP, scale=SCALE)
                    if j >= 0:
                        nc.vector.tensor_mul(
                            pt[:, c0:512], pt[:, c0:512],
                            mask_t[:, j * 512 + c0: (j + 1) * 512])
                    pts[kt] = (pt, c0)

                def emit_back(kt, m=m, qg=qg, nkt=nkt, yps=yps, dps=dps):
                    pt, c0 = pts.pop(kt)
                    nc.tensor.matmul(
                        yps[:, c0:512],
                        vsb[:, kt * QF + m * 128: kt * QF + m * 128 + 128],
                        pt[:, c0:512],
                        start=(kt == 0), stop=(kt == nkt - 1),
                        skip_group_check=True)
                    nc.tensor.matmul(
                        dps[:, c0:512], ones_t[:], pt[:, c0:512],
                        start=(kt == 0), stop=(kt == nkt - 1),
                        skip_group_check=True)

                for kt in range(nkt):
                    emit_front(kt)
                    if kt >= LOOKAHEAD:
                        emit_back(kt - LOOKAHEAD)
                for kt in range(max(0, nkt - LOOKAHEAD), nkt):
                    emit_back(kt)

                rec = np_.tile([128, 512], F32, name=f"rec{m}_{qg}", tag="rec")
                nc.vector.reciprocal(rec[:], dps[:])
                nc.vector.tensor_mul(
                    yhat[:, qh0 + qg * 512: qh0 + qg * 512 + 512], yps[:], rec[:])


def _body(nc, tc, S, xT, wq, wk, wv, wo, out,
          cos_t, sin_t, mask_t, ones_t, qr, kr, vsb, stages="ABC"):
    _phase1(nc, tc, S, xT, wq, wk, wv, cos_t, sin_t, qr, kr, vsb)

    if stages == "A":
        with tc.tile_pool(name="dbg", bufs=2) as dbg:
            for (src, r0) in ((qr, 0), (kr, 1)):
                t = dbg.tile([128, S], F32, name=f"dbgq{r0}", tag="dbg")
                nc.vector.tensor_copy(t[:], src[:, 0:S])
                nc.sync.dma_start(out[r0 * 128:(r0 + 1) * 128, 0:S], t[:])
            t = dbg.tile([128, QF], F32, name="dbgv", tag="dbgv")
            nc.vector.tensor_copy(t[:], vsb[:, 0:QF])
            nc.sync.dma_start(out[2 * 128:3 * 128, 0:QF], t[:])
        return

    _phase23(nc, tc, S, wo, out, mask_t, ones_t, qr, kr, vsb, stages)


def prep_in_maps(x, positions, Wqkv, Wout, S=2048):
    """Host-side shard/format. Returns per-core input dicts."""
    f16 = ml_dtypes.bfloat16 if False else np.float16

    # RoPE tables from positions (deinterleaved pair layout)
    inv_freq = 1.0 / (ROPE_BASE ** (np.arange(0, HD, 2, dtype=np.float64) / HD))  # [64]
    pos = np.asarray(positions).astype(np.float64)[:S]
    freq = pos[None, :] * inv_freq[:, None]          # [64, S]
    c = np.cos(freq).astype(np.float32)
    s = np.sin(freq).astype(np.float32)
    cosf = np.vstack([c, c]).astype(f16)              # [128, S]
    # swapped+sign-folded sin: out = P*cos + swap(P*sinf_sw)
    # rows 0:64 -> +s (will be added into odd rows), rows 64:128 -> -s
    sinf = np.vstack([s, -s]).astype(f16)             # [128, S]

    # diagonal causal masks M_j [128, 4*512]: key r (partition), query col c;
    # block j: cols [0,128j) dead, [128j,128j+128) triu (r<=c-128j), rest live
    dm = np.zeros((128, 4, 512), np.float32)
    for j in range(4):
        dm[:, j, 128 * j:128 * (j + 1)] = np.triu(np.ones((128, 128), np.float32))
        dm[:, j, 128 * (j + 1):] = 1.0
    dmasks = dm.reshape(128, 4 * 512).astype(f16)

    # per-head even/odd column permutation for q,k weights
    perm = np.concatenate([np.arange(0, HD, 2), np.arange(1, HD, 2)])

    in_maps = []
    for c_id in range(N_CORES):
        b, hh = c_id // 2, c_id % 2
        xTb = np.ascontiguousarray(x[b, :S, :].T).astype(f16)     # [D, S]
        f0 = hh * QF
        Wq = Wqkv[:, f0:f0 + QF]
        Wk = Wqkv[:, D + f0:D + f0 + QF]
        Wv = Wqkv[:, 2 * D + f0:2 * D + f0 + QF]
        # permute within each head for q, k
        Wqp = Wq.reshape(D, M, HD)[:, :, perm]        # [D, M, 128]
        Wkp = Wk.reshape(D, M, HD)[:, :, perm]
        # wq[m, p, k*128+f] = Wq[k*128+p, m*128+f]
        wq_p = np.ascontiguousarray(
            Wqp.reshape(KT, 128, M, HD).transpose(2, 1, 0, 3).reshape(M, 128, KT * 128)
        ).astype(f16)
        wk_p = np.ascontiguousarray(
            Wkp.reshape(KT, 128, M, HD).transpose(2, 1, 0, 3).reshape(M, 128, KT * 128)
        ).astype(f16)
        # wv[ncx, p, k*512+f] = Wv[k*128+p, ncx*512+f]
        wv_p = np.ascontiguousarray(
            Wv.reshape(KT, 128, 2, 512).transpose(2, 1, 0, 3).reshape(2, 128, KT * 512)
        ).astype(f16)
        # wo[p, (m*4+oc)*512+f] = Wout[f0 + m*128 + p, oc*512 + f]
        Woh = Wout[f0:f0 + QF, :]                     # [1024, 2048]
        wo_p = np.ascontiguousarray(
            Woh.reshape(M, 128, 4, 512).transpose(1, 0, 2, 3).reshape(128, M * 4 * 512)
        ).astype(f16)
        in_maps.append({
            "xT": xTb, "wq": wq_p, "wk": wk_p, "wv": wv_p, "wo": wo_p,
            "cosf": cosf, "sinf": sinf, "dmasks": dmasks,
        })
    return in_maps


def kernel(x, positions, mask, Wqkv, Wout):
    x = np.asarray(x, dtype=np.float32)
    Wqkv = np.asarray(Wqkv, dtype=np.float32)
    Wout = np.asarray(Wout, dtype=np.float32)
    S = x.shape[1]
    nc = build_nc(S=S)
    in_maps = prep_in_maps(x, positions, Wqkv, Wout, S=S)
    res = run_bass_kernel_spmd(nc, in_maps, core_ids=list(range(N_CORES)))
    outs = [res.results[c]["out"] for c in range(N_CORES)]
    full = np.stack([outs[2 * b] + outs[2 * b + 1] for b in range(B)], axis=0)
    return full.astype(np.float32)


# revision 11
# speedup vs baseline: 3.9078x; 3.4940x over previous
"""Causal self-attention (RoPE, B=4 S=2048 D=2048 H=16) on 8 Trainium2 cores.

Sharding: core c = 2*b + hh  ->  batch b = c // 2, head-half hh = c % 2
(8 heads per core).  Each core computes qkv projection for its heads,
attention, and a partial output projection over its 1024 y-features;
the host sums the two partials of each batch.

v2: fully SBUF-resident (no DRAM spills), fp16 inputs/compute with f32
PSUM accumulation, RoPE fused into PSUM eviction, causal diagonal-block
column trimming, software-pipelined attention inner loop.
"""
import sys

try:
    import concourse.bass as _chk  # noqa: F401
except ImportError:
    for p in ("/opt/trn_rl_repo", "/root/.axon_site/_ro/trn_rl_repo"):
        if p not in sys.path:
            sys.path.insert(0, p)

import math
import numpy as np
import ml_dtypes

import concourse.bass as bass
import concourse.tile as tile
from concourse import mybir
from concourse.bass_utils import run_bass_kernel_spmd

N_CORES = 8
B = 4
D = 2048
H = 16
HD = 128
M = 8                     # heads per core
QF = M * HD               # 1024 q/k/v features per core
KT = D // 128             # 16 contraction tiles
SH = 1024                 # phase-1 seq chunk
ROPE_BASE = 10000.0
F32 = mybir.dt.float32
F16 = mybir.dt.bfloat16
NPF16 = np.float16
EXP = mybir.ActivationFunctionType.Exp
SCALE = 1.0 / math.sqrt(HD)
TRIM = True     # trim dead query columns of causal diagonal blocks
SKIPGC = True   # skip_group_check on partial-column psum accumulation


def split_ctrl_waits(nc, maxw=1):
    """Walrus in this env can't encode >1 sem-wait on many instruction
    formats; move extras onto preceding same-engine NoOps."""
    nid = [0]
    for f in nc.m.functions:
        for b in f.blocks:
            new_insts = []
            for inst in b.instructions:
                si = inst.sync_info
                if si is not None and si.on_wait is not None and len(si.on_wait) > maxw:
                    waits = list(si.on_wait)
                    while len(waits) > maxw:
                        chunk, waits = waits[:maxw], waits[maxw:]
                        nid[0] += 1
                        nop = mybir.InstNoOp(
                            name=f"I-waitsplit-{nid[0]}",
                            ins=[], outs=[],
                            sync_info=mybir.SyncInfo(on_wait=chunk, on_update=[]),
                        )
                        nop.engine = inst.engine
                        new_insts.append(nop)
                    si.on_wait = waits
                new_insts.append(inst)
            b.instructions[:] = new_insts


def build_nc(S=2048, repeat=1, stages="ABC", split_waits=True):
    """One SPMD program; all 8 cores run it on different data."""
    nc = bass.Bass("TRN2", debug=False, num_devices=N_CORES)

    xT = nc.dram_tensor("xT", [D, S], F16, kind="ExternalInput")
    wq = nc.dram_tensor("wq", [M, 128, KT * 128], F16, kind="ExternalInput")
    wk = nc.dram_tensor("wk", [M, 128, KT * 128], F16, kind="ExternalInput")
    wv = nc.dram_tensor("wv", [2, 128, KT * 512], F16, kind="ExternalInput")
    wo = nc.dram_tensor("wo", [128, M * 4 * 512], F16, kind="ExternalInput")
    cosf = nc.dram_tensor("cosf", [128, S], F16, kind="ExternalInput")
    sinf = nc.dram_tensor("sinf", [128, S], F16, kind="ExternalInput")  # pre-swapped+sign-folded
    dmasks = nc.dram_tensor("dmasks", [128, 4 * 512], F16, kind="ExternalInput")
    out = nc.dram_tensor("out", [S, D], F32, kind="ExternalOutput")

    with tile.TileContext(nc) as tc:
        with tc.tile_pool(name="const", bufs=1) as constp:
            cos_t = constp.tile([128, S], F16, name="cos_t")
            sin_t = constp.tile([128, S], F16, name="sin_t")
            mask_t = constp.tile([128, 4 * 512], F16, name="mask_t")
            ones_t = constp.tile([128, 128], F16, name="ones_t")
            nc.sync.dma_start(cos_t[:], cosf[:])
            nc.sync.dma_start(sin_t[:], sinf[:])
            nc.sync.dma_start(mask_t[:], dmasks[:])
            nc.vector.memset(ones_t[:], 1.0)

            qr = constp.tile([128, M * S], F16, name="qr")
            kr = constp.tile([128, M * S], F16, name="kr")
            vsb = constp.tile([128, (S // 128) * QF], F16, name="vsb")
            # vsb col = st*1024 + f   (f = m*128 + hd)

            for _rep in range(repeat):
                _body(nc, tc, S, xT, wq, wk, wv, wo, out,
                      cos_t, sin_t, mask_t, ones_t, qr, kr, vsb, stages)

    if split_waits:
        split_ctrl_waits(nc)
    return nc


def _phase1(nc, tc, S, xT, wq, wk, wv, cos_t, sin_t, qr, kr, vsb):
    """QKV projection + fused RoPE, fp16 in, f32 psum, fp16 out."""
    NH = S // SH              # seq halves
    NCX = SH // 512           # 512-chunks per half

    with tc.tile_pool(name="p1", bufs=2) as xp, \
         tc.tile_pool(name="p1wv", bufs=1) as wvp, \
         tc.tile_pool(name="p1w", bufs=2) as wp, \
         tc.tile_pool(name="p1u", bufs=3) as up, \
         tc.tile_pool(name="p1ps", bufs=2, space="PSUM") as psp:

        for h in range(NH):
            xt = xp.tile([128, KT * SH], F16, name=f"xt{h}", tag="xt")
            for k in range(KT):
                nc.sync.dma_start(
                    xt[:, k * SH:(k + 1) * SH],
                    xT[k * 128:(k + 1) * 128, h * SH:(h + 1) * SH])

            # ---- q^T, k^T with fused RoPE (first: small weight loads) ----
            for w_dram, dst, tg in ((wq, qr, "q"), (wk, kr, "k")):
                for m in range(M):
                    wt = wp.tile([128, KT * 128], F16, name=f"w{tg}{h}_{m}", tag="wqk")
                    nc.sync.dma_start(wt[:], w_dram[m])
                    pos = [psp.tile([128, 512], F32, name=f"qp{tg}{h}_{m}_{c}", tag=f"qps{c}")
                           for c in range(NCX)]
                    for k in range(KT):
                        for c in range(NCX):
                            nc.tensor.matmul(
                                pos[c][:],
                                wt[:, k * 128:(k + 1) * 128],
                                xt[:, k * SH + c * 512: k * SH + c * 512 + 512],
                                start=(k == 0), stop=(k == KT - 1))
                    for c in range(NCX):
                        s0 = h * SH + c * 512     # seq offset
                        dslot = dst[:, m * S + s0: m * S + s0 + 512]
                        u = up.tile([128, 512], F16, name=f"u{tg}{h}_{m}_{c}", tag="u")
                        us = up.tile([128, 512], F16, name=f"us{tg}{h}_{m}_{c}", tag="us")
                        nc.vector.tensor_mul(u[:], pos[c][:], sin_t[:, s0:s0 + 512])
                        nc.vector.tensor_mul(dslot, pos[c][:], cos_t[:, s0:s0 + 512])
                        nc.vector.tensor_copy(us[0:64, :], u[64:128, :])
                        nc.vector.tensor_copy(us[64:128, :], u[0:64, :])
                        nc.vector.tensor_add(dslot, dslot, us[:])

            # ---- v: natural [seq, feat] ----
            for ncx in range(2):
                wvt = wvp.tile([128, KT * 512], F16, name=f"wv{h}_{ncx}", tag="wv")
                nc.sync.dma_start(wvt[:], wv[ncx])
                for st in range(SH // 128):
                    ps = psp.tile([128, 512], F32, name=f"vp{h}_{ncx}_{st}", tag="vps")
                    for k in range(KT):
                        nc.tensor.matmul(
                            ps[:],
                            xt[:, k * SH + st * 128: k * SH + st * 128 + 128],
                            wvt[:, k * 512:(k + 1) * 512],
                            start=(k == 0), stop=(k == KT - 1))
                    stg = h * (SH // 128) + st
                    nc.scalar.copy(vsb[:, stg * QF + ncx * 512: stg * QF + ncx * 512 + 512], ps[:])


def _phase23(nc, tc, S, wo, out, mask_t, ones_t, qr, kr, vsb, stages):
    NQ = S // 512
    with tc.tile_pool(name="p2", bufs=1) as cp:
        yhat = cp.tile([128, M * S], F16, name="yhat")
        wo_sb = cp.tile([128, M * 4 * 512], F16, name="wo_sb")
        nc.sync.dma_start(wo_sb[:], wo[:])

        _phase2(nc, tc, S, mask_t, ones_t, qr, kr, vsb, yhat)

        if stages == "AB":
            with tc.tile_pool(name="dbg2", bufs=2) as dbg:
                for r0 in range(2):
                    t = dbg.tile([128, S], F32, name=f"dbgy{r0}", tag="dbg")
                    nc.vector.tensor_copy(t[:], yhat[:, r0 * S:(r0 + 1) * S])
                    nc.sync.dma_start(out[r0 * 128:(r0 + 1) * 128, 0:S], t[:])
            return

        # ---------------- phase 3: output projection (partial) ----------------
        with tc.tile_pool(name="p3o", bufs=4) as cop, \
             tc.tile_pool(name="p3ps", bufs=2, space="PSUM") as cps:
            for st in range(S // 128):
                pos = [cps.tile([128, 512], F32, name=f"cpo{st}_{i}", tag=f"cpo{i}")
                       for i in range(4)]
                for m in range(M):
                    for oc in range(4):
                        nc.tensor.matmul(
                            pos[oc][:],
                            yhat[:, m * S + st * 128: m * S + st * 128 + 128],
                            wo_sb[:, (m * 4 + oc) * 512:(m * 4 + oc + 1) * 512],
                            start=(m == 0), stop=(m == M - 1))
                for oc in range(4):
                    ot = cop.tile([128, 512], F32, name=f"cot{st}_{oc}", tag="cot")
                    nc.scalar.copy(ot[:], pos[oc][:])
                    nc.sync.dma_start(
                        out[st * 128:(st + 1) * 128, oc * 512:(oc + 1) * 512], ot[:])


def _phase2(nc, tc, S, mask_t, ones_t, qr, kr, vsb, yhat):
    NQ = S // 512
    with tc.tile_pool(name="p2pt", bufs=4) as ptp, \
         tc.tile_pool(name="p2n", bufs=2) as np_, \
         tc.tile_pool(name="p2ps", bufs=3, space="PSUM") as sps_p, \
         tc.tile_pool(name="p2ac", bufs=2, space="PSUM") as acc_p:

        for m in range(M):
            qh0 = m * S
            for qg in range(NQ):
                nkt = 4 * qg + 4
                yps = acc_p.tile([128, 512], F32, name=f"yps{m}_{qg}", tag="yps")
                dps = acc_p.tile([128, 512], F32, name=f"dps{m}_{qg}", tag="dps")

                # software-pipelined: scores/exp run LOOKAHEAD blocks ahead
                # of the yps/dps accumulation matmuls.
                LOOKAHEAD = 2
                pts = {}

                def emit_front(kt, m=m, qg=qg, nkt=nkt, yps=yps, dps=dps):
                    j = kt - 4 * qg
                    c0 = 128 * j if (j >= 0 and TRIM) else 0
                    w = 512 - c0
                    sp = sps_p.tile([128, 512], F32, name=f"sp{m}_{qg}_{kt}", tag="sps")
                    nc.tensor.matmul(
                        sp[:, c0:512],
                        kr[:, qh0 + kt * 128: qh0 + kt * 128 + 128],
                        qr[:, qh0 + qg * 512 + c0: qh0 + qg * 512 + 512],
                        start=True, stop=True)
                    pt = ptp.tile([128, 512], F16, name=f"pt{m}_{qg}_{kt}", tag="pt")
                    nc.scalar.activation(pt[:, c0:512], sp[:, c0:512], EXP, scale=SCALE)
                    if j >= 0:
                        nc.vector.tensor_mul(
                            pt[:, c0:512], pt[:, c0:512],
                            mask_t[:, j * 512 + c0: (j + 1) * 512])
                    pts[kt] = (pt, c0)

                def emit_back(kt, m=m, qg=qg, nkt=nkt, yps=yps, dps=dps):
                    pt, c0 = pts.pop(kt)
                    nc.tensor.matmul(
                        yps[:, c0:512],
                        vsb[:, kt * QF + m * 128: kt * QF + m * 128 + 128],
                        pt[:, c0:512],
                        start=(kt == 0), stop=(kt == nkt - 1),
                        skip_group_check=SKIPGC)
                    nc.tensor.matmul(
                        dps[:, c0:512], ones_t[:], pt[:, c0:512],
                        start=(kt == 0), stop=(kt == nkt - 1),
                        skip_group_check=SKIPGC)

                for kt in range(nkt):
                    emit_front(kt)
                    if kt >= LOOKAHEAD:
                        emit_back(kt - LOOKAHEAD)
                for kt in range(max(0, nkt - LOOKAHEAD), nkt):
                    emit_back(kt)

                rec = np_.tile([128, 512], F32, name=f"rec{m}_{qg}", tag="rec")
                nc.vector.reciprocal(rec[:], dps[:])
                nc.vector.tensor_mul(
                    yhat[:, qh0 + qg * 512: qh0 + qg * 512 + 512], yps[:], rec[:])


def _body(nc, tc, S, xT, wq, wk, wv, wo, out,
          cos_t, sin_t, mask_t, ones_t, qr, kr, vsb, stages="ABC"):
    _phase1(nc, tc, S, xT, wq, wk, wv, cos_t, sin_t, qr, kr, vsb)

    if stages == "A":
        with tc.tile_pool(name="dbg", bufs=2) as dbg:
            for (src, r0) in ((qr, 0), (kr, 1)):
                t = dbg.tile([128, S], F32, name=f"dbgq{r0}", tag="dbg")
                nc.vector.tensor_copy(t[:], src[:, 0:S])
                nc.sync.dma_start(out[r0 * 128:(r0 + 1) * 128, 0:S], t[:])
            t = dbg.tile([128, QF], F32, name="dbgv", tag="dbgv")
            nc.vector.tensor_copy(t[:], vsb[:, 0:QF])
            nc.sync.dma_start(out[2 * 128:3 * 128, 0:QF], t[:])
        return

    _phase23(nc, tc, S, wo, out, mask_t, ones_t, qr, kr, vsb, stages)


def prep_in_maps(x, positions, Wqkv, Wout, S=2048):
    """Host-side shard/format. Returns per-core input dicts."""
    f16 = ml_dtypes.bfloat16

    # RoPE tables from positions (deinterleaved pair layout)
    inv_freq = 1.0 / (ROPE_BASE ** (np.arange(0, HD, 2, dtype=np.float64) / HD))  # [64]
    pos = np.asarray(positions).astype(np.float64)[:S]
    freq = pos[None, :] * inv_freq[:, None]          # [64, S]
    c = np.cos(freq).astype(np.float32)
    s = np.sin(freq).astype(np.float32)
    cosf = np.vstack([c, c]).astype(f16)              # [128, S]
    # swapped+sign-folded sin: out = P*cos + swap(P*sinf_sw)
    # rows 0:64 -> +s (will be added into odd rows), rows 64:128 -> -s
    sinf = np.vstack([s, -s]).astype(f16)             # [128, S]

    # diagonal causal masks M_j [128, 4*512]: key r (partition), query col c;
    # block j: cols [0,128j) dead, [128j,128j+128) triu (r<=c-128j), rest live
    dm = np.zeros((128, 4, 512), np.float32)
    for j in range(4):
        dm[:, j, 128 * j:128 * (j + 1)] = np.triu(np.ones((128, 128), np.float32))
        dm[:, j, 128 * (j + 1):] = 1.0
    dmasks = dm.reshape(128, 4 * 512).astype(f16)

    # per-head even/odd column permutation for q,k weights
    perm = np.concatenate([np.arange(0, HD, 2), np.arange(1, HD, 2)])

    in_maps = []
    for c_id in range(N_CORES):
        b, hh = c_id // 2, c_id % 2
        xTb = np.ascontiguousarray(x[b, :S, :].T).astype(f16)     # [D, S]
        f0 = hh * QF
        Wq = Wqkv[:, f0:f0 + QF]
        Wk = Wqkv[:, D + f0:D + f0 + QF]
        Wv = Wqkv[:, 2 * D + f0:2 * D + f0 + QF]
        # permute within each head for q, k
        Wqp = Wq.reshape(D, M, HD)[:, :, perm]        # [D, M, 128]
        Wkp = Wk.reshape(D, M, HD)[:, :, perm]
        # wq[m, p, k*128+f] = Wq[k*128+p, m*128+f]
        wq_p = np.ascontiguousarray(
            Wqp.reshape(KT, 128, M, HD).transpose(2, 1, 0, 3).reshape(M, 128, KT * 128)
        ).astype(f16)
        wk_p = np.ascontiguousarray(
            Wkp.reshape(KT, 128, M, HD).transpose(2, 1, 0, 3).reshape(M, 128, KT * 128)
        ).astype(f16)
        # wv[ncx, p, k*512+f] = Wv[k*128+p, ncx*512+f]
        wv_p = np.ascontiguousarray(
            Wv.reshape(KT, 128, 2, 512).transpose(2, 1, 0, 3).reshape(2, 128, KT * 512)
        ).astype(f16)
        # wo[p, (m*4+oc)*512+f] = Wout[f0 + m*128 + p, oc*512 + f]
        Woh = Wout[f0:f0 + QF, :]                     # [1024, 2048]
        wo_p = np.ascontiguousarray(
            Woh.reshape(M, 128, 4, 512).transpose(1, 0, 2, 3).reshape(128, M * 4 * 512)
        ).astype(f16)
        in_maps.append({
            "xT": xTb, "wq": wq_p, "wk": wk_p, "wv": wv_p, "wo": wo_p,
            "cosf": cosf, "sinf": sinf, "dmasks": dmasks,
        })
    return in_maps


def kernel(x, positions, mask, Wqkv, Wout):
    x = np.asarray(x, dtype=np.float32)
    Wqkv = np.asarray(Wqkv, dtype=np.float32)
    Wout = np.asarray(Wout, dtype=np.float32)
    S = x.shape[1]
    nc = build_nc(S=S)
    in_maps = prep_in_maps(x, positions, Wqkv, Wout, S=S)
    res = run_bass_kernel_spmd(nc, in_maps, core_ids=list(range(N_CORES)))
    outs = [res.results[c]["out"] for c in range(N_CORES)]
    full = np.stack([outs[2 * b] + outs[2 * b + 1] for b in range(B)], axis=0)
    return full.astype(np.float32)
